# revision 2
# baseline (speedup 1.0000x reference)
"""Trainium2 Bass kernel: LinearSelfAttentionTemporal (N,C,T,V)=(64,128,64,25).

Data-parallel over batch N across 8 NeuronCores (8 samples each).
Per sample the pipeline runs in the natural (C=128 partitions, L=T*V=1600
free) layout:
  - c_attn / c_proj 1x1 convs as PE matmuls contracting over C
  - cumulative sums via DVE tensor_tensor_scan along the free dim
  - softmax WITHOUT max-subtraction: logits = temp*sum_hd(wn) are in
    [0, 16*temp] (wn = wsq/denom <= 1 since denom is an inclusive cumsum),
    so exp() is safe in fp32; denom_bias adds a per-(n,h) constant to the
    logits and cancels exactly in softmax, so it is dropped.
  - softmax post-ops batched over all 8 samples as one (64, L) tile set
  - per-head (8 -> 128 partition) broadcasts via 0-stride DMA replication
Algebra: with Pi = softmax(tmp), A = cumsum(Pi)+1e-8,
  dots = cumsum(wsq*Pi)/A  =>  attn = 1/(1+dots) = A/D
  where D = 1e-8 + cumsum((wsq+1)*Pi)  (scan with data1=Pi fused add)
  y = -(w*Pi)*attn = -(w * (Pi*A)_bcast) / D   (minus folded into -Wp^T)
The reference denom clamp max(cumsum(wsq), 1e-12) is reproduced exactly by
the scan recurrence state=(wsq+state) max 1e-12 (error <= 1e-12 absolute).
"""
import os
import sys

import numpy as np

for _p in ("/opt/trn_rl_repo",):
    if _p not in sys.path and os.path.isdir(_p):
        sys.path.insert(0, _p)

import ml_dtypes
import concourse.bacc as bacc
import concourse.tile as tile
from concourse import mybir
from concourse.bass_utils import run_bass_kernel_spmd

F32 = mybir.dt.float32
BF16 = mybir.dt.bfloat16
FP16 = mybir.dt.float16
AOP = mybir.AluOpType
AFT = mybir.ActivationFunctionType

N, C, T, V = 64, 128, 64, 25
H, HD, L = 8, 16, T * V
NCORES = 8
NLOC = N // NCORES
G, GS = 2, NLOC // 2  # two groups of 4 samples
CHUNKS = [(0, 512), (512, 512), (1024, 512), (1536, 64)]  # psum-bank aligned
# halves of L for the 2-bank psum tiles: (offset, width, sub-chunks)
HALVES = [(0, 1024, [(0, 512), (512, 512)]), (1024, 576, [(0, 512), (512, 64)])]

DEFAULT_CFG = dict(
    wsq_eng="s",   # wsq epilogue: "s" scalar Square-act | "d" DVE w*w
    wn_eng="g",    # wn = wsq*rden: "g" gpsimd | "d" dve
    v2_eng="g",    # v2 = w*u_b:   "g" gpsimd | "d" dve
    p2m_eng="d",   # p2m = wsq*PiB
    y_mod=2,       # y = v2*rD on gpsimd when n % y_mod == y_rem else dve
    y_rem=1,
)


def _dt(name):
    return {"bf16": BF16, "f32": F32, "fp16": FP16}[name]


def _act_recip(nc, out, in_):
    """Scalar-engine Reciprocal activation (HW-verified ~1.2e-5 rel err for
    normal-range inputs; inputs here are clamped >= 1e-12)."""
    ins = [nc.scalar.lower_ap(in_)]
    for arg in (0.0, 1.0, 0.0):  # bias, scale, alpha immediates
        ins.append(mybir.ImmediateValue(dtype=mybir.dt.float32, value=arg))
    return nc.scalar.add_instruction(
        mybir.InstActivation(
            name=nc.get_next_instruction_name(),
            func=mybir.ActivationFunctionType.Reciprocal,
            ins=ins,
            outs=[nc.scalar.lower_ap(out)],
        )
    )


def build_nc(cfg=None):
    """Build and compile the per-core Bass program. Returns nc."""
    cfg = {**DEFAULT_CFG, **(cfg or {})}
    cfg["y_mod"] = int(cfg.get("y_mod", 2))
    cfg["y_rem"] = int(cfg.get("y_rem", 1))
    from contextlib import ExitStack

    nc = bacc.Bacc("TRN2", target_bir_lowering=False, debug=False)

    x_d = nc.dram_tensor("x16", (NLOC, C, L), FP16, kind="ExternalInput").ap()
    wat_d = nc.dram_tensor("wat16", (C, C), FP16, kind="ExternalInput").ap()
    wptn_d = nc.dram_tensor("wptn_bf", (C, C), BF16, kind="ExternalInput").ap()
    iden_d = nc.dram_tensor("iden16", (C, C), FP16, kind="ExternalInput").ap()
    ba_d = nc.dram_tensor("ba", (C, 1), F32, kind="ExternalInput").ap()
    bp_d = nc.dram_tensor("bp", (C, 1), F32, kind="ExternalInput").ap()
    m64_d = nc.dram_tensor("m64bf", (C, NLOC * 32), BF16, kind="ExternalInput").ap()
    sc_d = nc.dram_tensor("sc64", (64, 1), F32, kind="ExternalInput").ap()
    out_d = nc.dram_tensor("out16", (NLOC, C, L), FP16, kind="ExternalOutput").ap()

    with tile.TileContext(nc) as tc, ExitStack() as ctx:
        cons = ctx.enter_context(tc.tile_pool(name="consts", bufs=1))
        xpool = ctx.enter_context(tc.tile_pool(name="xp", bufs=NLOC))
        wpool = ctx.enter_context(tc.tile_pool(name="wp", bufs=NLOC))
        sqpool = ctx.enter_context(tc.tile_pool(name="sqp", bufs=NLOC))
        work = ctx.enter_context(tc.tile_pool(name="wk", bufs=2))
        soft = ctx.enter_context(tc.tile_pool(name="sf", bufs=1))
        opool = ctx.enter_context(tc.tile_pool(name="op", bufs=2))
        pspool = ctx.enter_context(tc.tile_pool(name="ps", bufs=1, space="PSUM"))

        wat_s = cons.tile([C, C], FP16)
        nc.sync.dma_start(wat_s[:], wat_d[:])
        wptn_s = cons.tile([C, C], BF16)
        nc.sync.dma_start(wptn_s[:], wptn_d[:])
        iden_s = cons.tile([C, C], FP16)
        nc.sync.dma_start(iden_s[:], iden_d[:])
        ba_s = cons.tile([C, 1], F32)
        nc.sync.dma_start(ba_s[:], ba_d[:])
        bp_s = cons.tile([C, 1], F32)
        nc.sync.dma_start(bp_s[:], bp_d[:])
        m64_s = cons.tile([C, NLOC * 32], BF16)
        nc.sync.dma_start(m64_s[:], m64_d[:])
        sc_s = cons.tile([64, 1], F32)
        nc.sync.dma_start(sc_s[:], sc_d[:])
        eps_c = cons.tile([C, 1], BF16)
        nc.gpsimd.memset(eps_c[:], 1e-12)
        epsC = eps_c[:].broadcast_to((C, L))
        z64 = cons.tile([64, 1], BF16)
        nc.gpsimd.memset(z64[:], 0.0)
        z64L = z64[:].broadcast_to((64, L))

        x_l = [None] * NLOC
        w_l = [None] * NLOC
        wsq_l = [None] * NLOC

        # softmax batch tiles (both groups)
        e_t = soft.tile([64, L], BF16)
        s64 = soft.tile([64, 1], F32)

        # preload all x up front (DMA overlaps compute)
        for n in range(NLOC):
            x_h = xpool.tile([C, L], FP16, tag="xh")
            nc.sync.dma_start(x_h[:], x_d[n])
            x_l[n] = x_h

        def phase_a(g):
            ptmp = pspool.tile([32, 2048], F32, tag="ptmp", bufs=1)
            for j in range(GS):
                n = g * GS + j
                x_h = x_l[n]
                w_t = wpool.tile([C, L], BF16, tag="w")
                w_l[n] = w_t
                wsq_t = sqpool.tile([C, L], BF16, tag="wsq")
                wsq_l[n] = wsq_t
                for (ho, hw, subs) in HALVES:
                    pw = pspool.tile([C, 1024], F32, tag="pw", bufs=1)
                    for (so, sw) in subs:
                        nc.tensor.matmul(
                            pw[:, so : so + sw],
                            wat_s[:],
                            x_h[:, ho + so : ho + so + sw],
                            start=True,
                            stop=True,
                        )
                    nc.scalar.activation(
                        w_t[:, ho : ho + hw], pw[:, 0:hw], AFT.Identity, bias=ba_s[:]
                    )
                    if cfg["wsq_eng"] == "s":
                        nc.scalar.activation(
                            wsq_t[:, ho : ho + hw], pw[:, 0:hw], AFT.Square, bias=ba_s[:]
                        )
                if cfg["wsq_eng"] == "d":
                    nc.vector.tensor_tensor(wsq_t[:], w_t[:], w_t[:], AOP.mult)

                denom = work.tile([C, L], BF16, tag="den", bufs=2)
                nc.vector.tensor_tensor_scan(
                    denom[:], wsq_t[:], epsC, 0.0, AOP.add, AOP.max
                )
                rden = work.tile([C, L], BF16, tag="rden", bufs=2)
                _act_recip(nc, rden[:], denom[:])
                wn = work.tile([C, L], BF16, tag="wn", bufs=2)
                eng = nc.gpsimd if cfg["wn_eng"] == "g" else nc.vector
                eng.tensor_tensor(wn[:], wsq_t[:], rden[:], AOP.mult)

                for k, (o, cw) in enumerate(CHUNKS):
                    nc.tensor.matmul(
                        ptmp[0:32, k * 512 : k * 512 + cw],
                        m64_s[:, n * 32 : (n + 1) * 32],
                        wn[:, o : o + cw],
                        start=(j == 0),
                        stop=(j == GS - 1),
                    )
            # softmax exp straight from psum; no max subtraction (logits<=16*temp)
            r0, r1 = g * 32, (g + 1) * 32
            nc.scalar.activation(
                e_t[r0:r1, :],
                ptmp[0:32, 0:L],
                AFT.Exp,
                scale=sc_s[r0:r1, :],
                accum_out=s64[r0:r1, :],
            )

        def softmax_batch():
            rs = soft.tile([64, 1], F32)
            nc.vector.reciprocal(rs[:], s64[:])
            s8 = soft.tile([64, 1], F32)
            nc.vector.tensor_scalar_mul(s8[:], s64[:], 1e-8)
            cumE = soft.tile([64, L], BF16)
            nc.vector.tensor_tensor_scan(
                cumE[:], e_t[:], z64L, 0.0, AOP.add, AOP.add
            )
            Pi = soft.tile([64, L], BF16)
            nc.vector.tensor_scalar_mul(Pi[:], e_t[:], rs[:])
            u = soft.tile([64, L], BF16)
            nc.vector.tensor_scalar(u[:], cumE[:], s8[:], rs[:], AOP.add, AOP.mult)
            nc.vector.tensor_tensor(u[:], u[:], Pi[:], AOP.mult)
            return Pi, u

        def phase_c(g, Pi, u):
            for j in range(GS):
                n = g * GS + j
                r = g * 32 + 8 * j
                PiB = work.tile([C, L], BF16, tag="pib", bufs=3)
                nc.sync.dma_start(
                    PiB[:], Pi[r : r + 8, :].unsqueeze(1).broadcast_to((8, HD, L))
                )
                u_b = work.tile([C, L], BF16, tag="ub", bufs=3)
                nc.sync.dma_start(
                    u_b[:], u[r : r + 8, :].unsqueeze(1).broadcast_to((8, HD, L))
                )

                p2m = work.tile([C, L], BF16, tag="p2m", bufs=2)
                eng = nc.gpsimd if cfg["p2m_eng"] == "g" else nc.vector
                eng.tensor_tensor(p2m[:], wsq_l[n][:], PiB[:], AOP.mult)
                D_t = work.tile([C, L], BF16, tag="D", bufs=2)
                nc.vector.tensor_tensor_scan(
                    D_t[:], p2m[:], PiB[:], 1e-8, AOP.add, AOP.add
                )
                rD = work.tile([C, L], BF16, tag="rD", bufs=2)
                _act_recip(nc, rD[:], D_t[:])
                v2 = work.tile([C, L], BF16, tag="v2", bufs=2)
                eng = nc.gpsimd if cfg["v2_eng"] == "g" else nc.vector
                eng.tensor_tensor(v2[:], w_l[n][:], u_b[:], AOP.mult)
                y_t = work.tile([C, L], BF16, tag="y", bufs=2)
                eng = nc.gpsimd if (n % cfg["y_mod"]) == cfg["y_rem"] else nc.vector
                eng.tensor_tensor(y_t[:], v2[:], rD[:], AOP.mult)

                out_sb = opool.tile([C, L], FP16, tag="outsb")
                for (ho, hw, subs) in HALVES:
                    pj = pspool.tile([C, 1024], F32, tag="pj", bufs=1)
                    for (so, sw) in subs:
                        nc.tensor.matmul(
                            pj[:, so : so + sw],
                            wptn_s[:],
                            y_t[:, ho + so : ho + so + sw],
                            start=True,
                            stop=False,
                        )
                    for (so, sw) in subs:
                        nc.tensor.matmul(
                            pj[:, so : so + sw],
                            iden_s[:],
                            x_l[n][:, ho + so : ho + so + sw],
                            start=False,
                            stop=True,
                        )
                    nc.scalar.activation(
                        out_sb[:, ho : ho + hw], pj[:, 0:hw], AFT.Relu, bias=bp_s[:]
                    )
                nc.sync.dma_start(out_d[n], out_sb[:])

        phase_a(0)
        phase_a(1)
        Pi, u = softmax_batch()
        phase_c(0, Pi, u)
        phase_c(1, Pi, u)

    nc.compile()
    return nc


def make_core_inputs(inputs, cfg=None):
    """Host-side prep: returns (shared_map, per_core_x_list)."""
    x = np.asarray(inputs["x"], np.float32)  # (N,C,T,V)
    Wa = np.asarray(inputs["Wa"], np.float32)
    ba = np.asarray(inputs["ba"], np.float32)
    Wp = np.asarray(inputs["Wp"], np.float32)
    bp = np.asarray(inputs["bp"], np.float32)
    temp = np.asarray(inputs["temp"], np.float32).reshape(H)
    # denom_bias adds a per-(n,h) constant to the softmax logits -> cancels.

    assert np.all(temp > 0), "kernel assumes temp > 0"
    assert temp.max() * 16.0 < 80.0, "kernel assumes exp(16*temp) fits fp32"

    xr = np.ascontiguousarray(x.reshape(N, C, L).astype(np.float16))
    wat16 = np.ascontiguousarray(Wa.T).astype(np.float16)
    wptn_bf = np.ascontiguousarray((-Wp.T)).astype(ml_dtypes.bfloat16)
    iden16 = np.eye(C, dtype=np.float16)
    m64 = np.zeros((C, NLOC * 32), np.float32)
    cc = np.arange(C)
    for n in range(NLOC):
        m64[cc, n * 32 + 8 * (n % GS) + cc // HD] = 1.0
    m64bf = m64.astype(ml_dtypes.bfloat16)
    pp = np.arange(64)
    sc64 = temp[pp % 8].reshape(64, 1).astype(np.float32)

    shared = dict(
        wat16=wat16,
        wptn_bf=wptn_bf,
        iden16=iden16,
        ba=ba.reshape(C, 1),
        bp=bp.reshape(C, 1),
        m64bf=m64bf,
        sc64=sc64,
    )
    xs = [np.ascontiguousarray(xr[i * NLOC : (i + 1) * NLOC]) for i in range(NCORES)]
    return shared, xs


_NC_CACHE = {}


def kernel(**inputs):
    cfg_key = "default"
    if cfg_key not in _NC_CACHE:
        _NC_CACHE[cfg_key] = build_nc()
    nc = _NC_CACHE[cfg_key]
    shared, xs = make_core_inputs(inputs)
    in_maps = [dict(shared, x16=xs[i]) for i in range(NCORES)]
    res = run_bass_kernel_spmd(nc, in_maps, core_ids=list(range(NCORES)))
    out = np.concatenate([res.results[i]["out16"] for i in range(NCORES)], axis=0)
    return out.reshape(N, C, T, V).astype(np.float32)


if __name__ == "__main__":
    rng = np.random.default_rng(0)
    demo = dict(
        x=rng.standard_normal((N, C, T, V)).astype(np.float32),
        Wa=rng.standard_normal((C, C)).astype(np.float32) / np.sqrt(C),
        ba=rng.standard_normal((C,)).astype(np.float32) * 0.01,
        Wp=rng.standard_normal((C, C)).astype(np.float32) / np.sqrt(C),
        bp=rng.standard_normal((C,)).astype(np.float32) * 0.01,
        temp=np.ones((H, 1), np.float32),
        denom_bias=np.zeros((H, 1, 1), np.float32),
    )
    o = kernel(**demo)
    print("out", o.shape, o.dtype, float(np.abs(o).max()))
